# revision 10
# baseline (speedup 1.0000x reference)
"""Trainium2 Bass kernel for nn_ConcatenationAggregator.

For each review r:
    out[r] = relu(concat(review_vecs[r],
                         user_vecs[adj_u[r]][perm_u],
                         item_vecs[adj_i[r]][perm_i]) @ W)

Strategy (pure data-parallel over reviews, 8 NeuronCores):
  - Feature permutations are folded into W on the host (192x64 constant).
  - Row gathers use the GPSIMD `dma_gather` ucode (int16 indices, <=1024
    indices per call).  Since the tables exceed 32768 rows, the host sorts
    each core's reviews into 8 groups by (user-table 32K chunk, item-table
    32K chunk) so that rebased indices fit int16.  The group sort is just a
    relabeling of which review each (partition, column) slot processes; the
    host un-permutes the output.
  - The review stream is host-transposed into a feature-major, packed
    128-partition layout so it feeds the PE rhs directly; the output is
    produced transposed from PSUM and unpacked on the host.
  - Gathered rows are PE-transposed (user sub-tile -> PSUM partitions 0:64,
    item -> 64:128) giving a K=128 stacked rhs so one matmul covers the
    user+item contribution; a second K=64 matmul adds the review term.
  - This toolchain build enforces ONE sync-wait slot per instruction, so
    the emission order is software-pipelined (matmuls of chunk t before the
    transposes of chunk t+1, relus of chunk t after the copies of chunk
    t+1), discarded "header" transposes absorb gather-DMA waits, and the
    kernel-tail drain is split into single-wait drains.
"""

import types

import numpy as np

import concourse.bacc as bacc
import concourse.bass as bass
import concourse.mybir as mybir
import concourse.tile as tile
from concourse.bass_utils import run_bass_kernel_spmd
from concourse.masks import make_identity
from concourse.vector_clock import ScopedClock, VectorClock

F32 = mybir.dt.float32
I16 = mybir.dt.int16

N_CORES = 8
D = 64
SUB = 128                  # reviews per sub-tile
MAX_S = 8                  # sub-tiles per chunk (<=1024 gather indices)
TCH = 32768                # table chunk (int16 index range)

N_REVIEWS = 1_000_000
N_USERS = 100_000
N_ITEMS = 50_000
RPC = N_REVIEWS // N_CORES


def _split_drain_and_barrier(self, tick_clock, wait_clock):
    """Replacement for TileContext._drain_and_barrier: the stock tail drain
    waits on every live proc semaphore at once, which overflows this
    toolchain's one-sync-wait-per-instruction limit.  Emit one drain per
    semaphore instead."""
    gc = tick_clock.global_clock
    ticks = list(gc)
    idxs = [i for i, t in enumerate(ticks) if t > 0]
    for i in idxs:
        sub = [0] * len(ticks)
        sub[i] = ticks[i]
        drain_inst = self.nc.sync.drain()
        wait_clock.add_sem_waits(
            drain_inst.ins, ScopedClock({None: VectorClock(sub)}))
    if not idxs:
        drain_inst = self.nc.sync.drain()
        wait_clock.add_sem_waits(
            drain_inst.ins, ScopedClock({None: VectorClock(ticks)}))
    self.nc.all_engine_barrier()
    assert self.sems is not None
    popped = self.nc._tile_sem_poison_stack.pop()
    assert popped is self._sem_poison
    self.nc.clear_and_free_semaphores(list(self.sems.allocated().values()))
    self.nc.all_engine_barrier()


def _chunk_list(s_per_group):
    """[(group, s_subtiles, row_base_slots, idxcol_base), ...] — shared by
    host packing and device program.  s values are even, <= MAX_S."""
    chunks = []
    row = 0
    col = 0
    for g, sg in enumerate(s_per_group):
        left = sg
        while left > 0:
            s = min(MAX_S, left)
            chunks.append((g, s, row, col))
            row += s * SUB
            col += s * 8
            left -= s
    return chunks


def _build_program(chunks, n_users, n_items):
    nc = bacc.Bacc("TRN2", target_bir_lowering=False, debug=False,
                   enable_asserts=False)
    padtot = sum(s for (_, s, _, _) in chunks) * SUB
    icols = padtot // 16

    rt_d = nc.dram_tensor("rt", [64, padtot], F32, kind="ExternalInput")
    uidx_d = nc.dram_tensor("uidx", [128, icols], I16, kind="ExternalInput")
    iidx_d = nc.dram_tensor("iidx", [128, icols], I16, kind="ExternalInput")
    tblu_d = nc.dram_tensor("tblu", [n_users, D], F32, kind="ExternalInput")
    tbli_d = nc.dram_tensor("tbli", [n_items, D], F32, kind="ExternalInput")
    w1_d = nc.dram_tensor("w1", [64, 64], F32, kind="ExternalInput")
    w2_d = nc.dram_tensor("w2p", [64, 64], F32, kind="ExternalInput")
    w3_d = nc.dram_tensor("w3p", [64, 64], F32, kind="ExternalInput")
    out_d = nc.dram_tensor("out2", [64, padtot], F32, kind="ExternalOutput")

    T = len(chunks)

    with tile.TileContext(nc) as tc:
        tc._drain_and_barrier = types.MethodType(_split_drain_and_barrier, tc)
        with tc.tile_pool(name="const", bufs=1) as constp, \
             tc.tile_pool(name="rtp", bufs=3) as rtp, \
             tc.tile_pool(name="gup", bufs=3) as gup, \
             tc.tile_pool(name="gip", bufs=3) as gip, \
             tc.tile_pool(name="outp", bufs=3) as outp, \
             tc.tile_pool(name="utp", bufs=3) as utp, \
             tc.tile_pool(name="itp", bufs=3) as itp, \
             tc.tile_pool(name="scr", bufs=1, space="PSUM") as scrp, \
             tc.tile_pool(name="tpp", bufs=4, space="PSUM") as tpp, \
             tc.tile_pool(name="mmp", bufs=3, space="PSUM") as mmp:

            ident = constp.tile([128, 128], F32)
            make_identity(nc, ident[:])
            w1_t = constp.tile([64, 64], F32)
            nc.sync.dma_start(out=w1_t[:], in_=w1_d.ap()[:, :])
            w2_t = constp.tile([64, 64], F32)
            nc.sync.dma_start(out=w2_t[:], in_=w2_d.ap()[:, :])
            w3_t = constp.tile([64, 64], F32)
            nc.sync.dma_start(out=w3_t[:], in_=w3_d.ap()[:, :])
            uidx_t = constp.tile([128, icols], I16)
            nc.sync.dma_start(out=uidx_t[:], in_=uidx_d.ap()[:, :])
            iidx_t = constp.tile([128, icols], I16)
            nc.sync.dma_start(out=iidx_t[:], in_=iidx_d.ap()[:, :])
            dummy_sb = constp.tile([64, 128], F32)
            pscr = constp.tile([16, 16], I16)

            scratch = scrp.tile([64, 128], F32)
            # PE warmups: observe the identity (Pool) and weight-load (HWDGE)
            # semaphores with one wait each.
            nc.tensor.transpose(out=scratch[:], in_=ident[:, 0:64],
                                identity=ident[:])
            nc.tensor.matmul(out=scratch[:, 0:64], lhsT=w2_t[:],
                             rhs=w2_t[:], start=True, stop=True)
            nc.tensor.matmul(out=scratch[:, 0:64], lhsT=w3_t[:],
                             rhs=w3_t[:], start=True, stop=True)
            nc.tensor.matmul(out=scratch[:, 0:64], lhsT=w1_t[:],
                             rhs=w1_t[:], start=True, stop=True)
            # Pool warmups: observe the index-table loads.
            nc.gpsimd.tensor_copy(out=pscr[:, :], in_=uidx_t[0:16, 0:16])
            nc.gpsimd.tensor_copy(out=pscr[:, :], in_=iidx_t[0:16, 0:16])

            nreg = {}
            for (_, s, _, _) in chunks:
                if s not in nreg:
                    nreg[s] = nc.gpsimd.to_reg(s * SUB)

            ubase = [g // 2 * TCH for g in range(8)]
            usize = [min(TCH, n_users - b) for b in ubase]
            ibase = [g % 2 * TCH for g in range(8)]
            isize = [min(TCH, n_items - b) for b in ibase]

            rt_tiles = [None] * T
            gu_tiles = [None] * T
            gi_tiles = [None] * T
            ui_tiles = [None] * T
            ps_tiles = [None] * T
            o_tiles = [None] * T

            def issue_loads(t):
                g, s, row, col = chunks[t]
                rt_t = rtp.tile([64, MAX_S * 128], F32, tag="rt")
                nc.sync.dma_start(
                    out=rt_t[:, :s * 128],
                    in_=rt_d.ap()[:, row: row + s * 128])
                gu_t = gup.tile([128, MAX_S * 64], F32, tag="gu")
                nc.gpsimd.dma_gather(
                    out_ap=gu_t[:, :s * 64].rearrange("p (n d) -> p n d", d=64),
                    in_ap=tblu_d.ap()[ubase[g]:ubase[g] + usize[g], :],
                    idxs_ap=uidx_t[:, col:col + s * 8],
                    num_idxs=s * SUB, num_idxs_reg=nreg[s], elem_size=64)
                gi_t = gip.tile([128, MAX_S * 64], F32, tag="gi")
                nc.gpsimd.dma_gather(
                    out_ap=gi_t[:, :s * 64].rearrange("p (n d) -> p n d", d=64),
                    in_ap=tbli_d.ap()[ibase[g]:ibase[g] + isize[g], :],
                    idxs_ap=iidx_t[:, col:col + s * 8],
                    num_idxs=s * SUB, num_idxs_reg=nreg[s], elem_size=64)
                rt_tiles[t], gu_tiles[t], gi_tiles[t] = rt_t, gu_t, gi_t

            def issue_transposes(t):
                _, s, _, _ = chunks[t]
                gu_t, gi_t = gu_tiles[t], gi_tiles[t]
                # Discarded header transposes absorb the two gather waits.
                nc.tensor.transpose(out=scratch[:], in_=gu_t[:, 0:64],
                                    identity=ident[:])
                nc.tensor.transpose(out=scratch[:], in_=gi_t[:, 0:64],
                                    identity=ident[:])
                ut_t = utp.tile([64, MAX_S * 128], F32, tag="ut")
                it_t = itp.tile([64, MAX_S * 128], F32, tag="it")
                for g4 in range((s + 3) // 4):
                    w = min(4, s - g4 * 4)
                    tpu = tpp.tile([64, 512], F32, tag="tp")
                    for jj in range(w):
                        j = g4 * 4 + jj
                        nc.tensor.transpose(
                            out=tpu[:, jj * 128:(jj + 1) * 128],
                            in_=gu_t[:, j * 64:(j + 1) * 64],
                            identity=ident[:])
                    nc.vector.tensor_copy(
                        out=ut_t[:, g4 * 512:g4 * 512 + w * 128],
                        in_=tpu[:, :w * 128])
                    tpi = tpp.tile([64, 512], F32, tag="tp")
                    for jj in range(w):
                        j = g4 * 4 + jj
                        nc.tensor.transpose(
                            out=tpi[:, jj * 128:(jj + 1) * 128],
                            in_=gi_t[:, j * 64:(j + 1) * 64],
                            identity=ident[:])
                    nc.vector.tensor_copy(
                        out=it_t[:, g4 * 512:g4 * 512 + w * 128],
                        in_=tpi[:, :w * 128])
                ui_tiles[t] = (ut_t, it_t)

            def issue_matmuls(t):
                _, s, _, _ = chunks[t]
                n = s * 64
                pss = []
                rt_t = rt_tiles[t]
                ut_t, it_t = ui_tiles[t]
                for q in range(2):
                    ps = mmp.tile([64, 512], F32, tag="mm")
                    ps_s = ps[:, :n]
                    nc.tensor.matmul(out=ps_s, lhsT=w2_t[:],
                                     rhs=ut_t[:, q * n:(q + 1) * n],
                                     start=True, stop=False)
                    nc.tensor.matmul(out=ps_s, lhsT=w3_t[:],
                                     rhs=it_t[:, q * n:(q + 1) * n],
                                     start=False, stop=False)
                    nc.tensor.matmul(out=ps_s, lhsT=w1_t[:],
                                     rhs=rt_t[:, q * n:(q + 1) * n],
                                     start=False, stop=True)
                    pss.append(ps)
                ps_tiles[t] = pss

            def issue_relus(t):
                _, s, _, _ = chunks[t]
                n = s * 64
                pss = ps_tiles[t]
                o_t = outp.tile([64, MAX_S * 128], F32, tag="o")
                for q in range(2):
                    nc.vector.tensor_scalar_max(
                        out=o_t[:, q * n:(q + 1) * n],
                        in0=pss[q][:, :n], scalar1=0.0)
                o_tiles[t] = o_t

            def issue_store(t):
                _, s, row, _ = chunks[t]
                nc.sync.dma_start(
                    out=out_d.ap()[:, row: row + s * 128],
                    in_=o_tiles[t][:, :s * 128])

            # Software-pipelined emission (see module docstring).
            issue_loads(0)
            issue_transposes(0)
            for t in range(T):
                if t + 1 < T:
                    issue_loads(t + 1)
                issue_matmuls(t)
                if t + 1 < T:
                    issue_transposes(t + 1)
                else:
                    # Dummy PE op after the last matmuls + a DVE observer so
                    # the last relus elide their PE wait.
                    nc.tensor.transpose(out=scratch[:], in_=ident[:, 0:64],
                                        identity=ident[:])
                    nc.vector.tensor_copy(out=dummy_sb[:], in_=scratch[:])
                issue_relus(t)
                issue_store(t)
    nc.finalize()
    return nc


_PROGRAM_CACHE: dict = {}


def _get_program(chunk_key, n_users, n_items):
    key = (chunk_key, n_users, n_items)
    if key not in _PROGRAM_CACHE:
        _PROGRAM_CACHE[key] = (
            _build_program(_chunk_list(list(chunk_key)), n_users, n_items))
    return _PROGRAM_CACHE[key]


def _pack_rt(rv_sorted, chunks):
    """[PADTOT, 64] sorted/padded reviews -> [64, PADTOT] feature-major."""
    return np.ascontiguousarray(rv_sorted.T)


def _unpack_out(o2, chunks):
    """[64, PADTOT] transposed output -> [PADTOT, 64]."""
    return np.ascontiguousarray(o2.T)


def _wrap_idx(flat_sorted, chunks):
    """Rebased int16 indices [PADTOT] -> [128, PADTOT//16] in dma_gather's
    wrapped layout: per chunk block [128, 8*s] with block[p, m] =
    flat[m*16 + p%16], replicated across the 8 16-partition groups."""
    cols = []
    for (_, s, row, _) in chunks:
        blk = flat_sorted[row:row + s * SUB].reshape(s * 8, 16).T  # [16, 8s]
        cols.append(np.tile(blk, (8, 1)))
    return np.ascontiguousarray(np.concatenate(cols, axis=1))


def _run(review_vecs, user_vecs, item_vecs, W,
         review_user_adj, review_item_adj, perm_u, perm_i,
         n_cores, rpc):
    n_users = user_vecs.shape[0]
    n_items = item_vecs.shape[0]

    W = np.asarray(W, np.float32)
    W1 = np.ascontiguousarray(W[0:64])
    W2 = W[64:128]
    W3 = W[128:192]
    perm_u = np.asarray(perm_u, np.int64)
    perm_i = np.asarray(perm_i, np.int64)
    W2p = np.empty_like(W2)
    W2p[perm_u] = W2
    W3p = np.empty_like(W3)
    W3p[perm_i] = W3
    W2p = np.ascontiguousarray(W2p)
    W3p = np.ascontiguousarray(W3p)

    user_vecs = np.ascontiguousarray(np.asarray(user_vecs, np.float32))
    item_vecs = np.ascontiguousarray(np.asarray(item_vecs, np.float32))
    review_vecs = np.asarray(review_vecs, np.float32)
    au_all = np.asarray(review_user_adj, np.int64)
    ai_all = np.asarray(review_item_adj, np.int64)

    # Group each core's reviews by (user 32K chunk, item 32K chunk).
    per_core = []
    s_max = np.zeros(8, np.int64)
    for c in range(n_cores):
        lo = c * rpc
        au = au_all[lo:lo + rpc]
        ai = ai_all[lo:lo + rpc]
        grp = (au // TCH) * 2 + (ai // TCH)
        order = np.argsort(grp, kind="stable")
        counts = np.bincount(grp, minlength=8)
        per_core.append((order, counts))
        s_max = np.maximum(s_max, -(-counts // SUB))
    # shared chunk structure: even sub-tile counts per group
    s_per_group = [int(s + (s % 2)) for s in s_max]
    chunk_key = tuple(s_per_group)
    chunks = _chunk_list(s_per_group)
    padtot = sum(s for (_, s, _, _) in chunks) * SUB

    nc = _get_program(chunk_key, n_users, n_items)

    in_maps = []
    slotmaps = []
    for c in range(n_cores):
        lo = c * rpc
        au = au_all[lo:lo + rpc]
        ai = ai_all[lo:lo + rpc]
        order, counts = per_core[c]
        grp_sorted_bounds = np.cumsum(counts)
        slotmap = np.full(padtot, -1, np.int64)
        row = 0
        start = 0
        for g in range(8):
            cnt = int(counts[g])
            ids = order[start:start + cnt]
            slotmap[row:row + cnt] = ids
            start += cnt
            row += s_per_group[g] * SUB
        valid = slotmap >= 0
        sl = np.where(valid, slotmap, 0)

        rv_sorted = np.where(valid[:, None],
                             review_vecs[lo:lo + rpc][sl], 0.0).astype(np.float32)
        slot_g = np.repeat(np.arange(8), np.array(s_per_group) * SUB)
        u_reb = np.where(valid, au[sl] - (slot_g // 2) * TCH, 0).astype(np.int16)
        i_reb = np.where(valid, ai[sl] - (slot_g % 2) * TCH, 0).astype(np.int16)

        in_maps.append({
            "rt": _pack_rt(rv_sorted, chunks),
            "uidx": _wrap_idx(u_reb, chunks),
            "iidx": _wrap_idx(i_reb, chunks),
            "tblu": user_vecs,
            "tbli": item_vecs,
            "w1": W1,
            "w2p": W2p,
            "w3p": W3p,
        })
        slotmaps.append((slotmap, valid))

    res = run_bass_kernel_spmd(nc, in_maps, core_ids=list(range(n_cores)))

    out = np.empty((n_cores * rpc, 64), np.float32)
    for c in range(n_cores):
        o2 = np.asarray(res.results[c]["out2"], np.float32)
        out_sorted = _unpack_out(o2, chunks)
        slotmap, valid = slotmaps[c]
        out[c * rpc + slotmap[valid]] = out_sorted[valid]
    return out


def kernel(**inputs) -> np.ndarray:
    return _run(
        inputs["review_vecs"], inputs["user_vecs"], inputs["item_vecs"],
        inputs["W"], inputs["review_user_adj"], inputs["review_item_adj"],
        inputs["perm_u"], inputs["perm_i"],
        n_cores=N_CORES, rpc=RPC)


# revision 13
# speedup vs baseline: 67.9631x; 67.9631x over previous
"""Trainium2 Bass kernel for nn_ConcatenationAggregator.

For each review r:
    out[r] = relu(concat(review_vecs[r],
                         user_vecs[adj_u[r]][perm_u],
                         item_vecs[adj_i[r]][perm_i]) @ W)

Strategy (pure data-parallel over reviews, 8 NeuronCores):
  - Feature permutations are folded into W on the host (192x64 constant).
  - Row gathers use the GPSIMD `dma_gather` ucode (int16 indices, <=1024
    indices per call).  Since the tables exceed 32768 rows, the host sorts
    each core's reviews into 8 groups by (user-table 32K chunk, item-table
    32K chunk) so that rebased indices fit int16.  The group sort is just a
    relabeling of which review each (partition, column) slot processes; the
    host un-permutes the output.
  - The review stream is host-transposed into a feature-major, packed
    128-partition layout so it feeds the PE rhs directly; the output is
    produced transposed from PSUM and unpacked on the host.
  - Gathered rows are PE-transposed (user sub-tile -> PSUM partitions 0:64,
    item -> 64:128) giving a K=128 stacked rhs so one matmul covers the
    user+item contribution; a second K=64 matmul adds the review term.
  - This toolchain build enforces ONE sync-wait slot per instruction, so
    the emission order is software-pipelined (matmuls of chunk t before the
    transposes of chunk t+1, relus of chunk t after the copies of chunk
    t+1), discarded "header" transposes absorb gather-DMA waits, and the
    kernel-tail drain is split into single-wait drains.
"""

import os
import types

import numpy as np

import concourse.bacc as bacc
import concourse.bass as bass
import concourse.mybir as mybir
import concourse.tile as tile
from concourse.bass_utils import run_bass_kernel_spmd
from concourse.masks import make_identity
from concourse.vector_clock import ScopedClock, VectorClock

F32 = mybir.dt.float32
I16 = mybir.dt.int16

N_CORES = 8
D = 64
SUB = 128                  # reviews per sub-tile
MAX_S = 8                  # sub-tiles per chunk (<=1024 gather indices)
TCH = 32768                # table chunk (int16 index range)

N_REVIEWS = 1_000_000
N_USERS = 100_000
N_ITEMS = 50_000
RPC = N_REVIEWS // N_CORES


def _split_drain_and_barrier(self, tick_clock, wait_clock):
    """Replacement for TileContext._drain_and_barrier: the stock tail drain
    waits on every live proc semaphore at once, which overflows this
    toolchain's one-sync-wait-per-instruction limit.  Emit one drain per
    semaphore instead."""
    gc = tick_clock.global_clock
    ticks = list(gc)
    idxs = [i for i, t in enumerate(ticks) if t > 0]
    for i in idxs:
        sub = [0] * len(ticks)
        sub[i] = ticks[i]
        drain_inst = self.nc.sync.drain()
        wait_clock.add_sem_waits(
            drain_inst.ins, ScopedClock({None: VectorClock(sub)}))
    if not idxs:
        drain_inst = self.nc.sync.drain()
        wait_clock.add_sem_waits(
            drain_inst.ins, ScopedClock({None: VectorClock(ticks)}))
    self.nc.all_engine_barrier()
    assert self.sems is not None
    popped = self.nc._tile_sem_poison_stack.pop()
    assert popped is self._sem_poison
    self.nc.clear_and_free_semaphores(list(self.sems.allocated().values()))
    self.nc.all_engine_barrier()


def _chunk_list(s_per_group):
    """[(group, s_subtiles, row_base_slots, idxcol_base), ...] — shared by
    host packing and device program.  s values are even, <= MAX_S."""
    chunks = []
    row = 0
    col = 0
    for g, sg in enumerate(s_per_group):
        left = sg
        while left > 0:
            s = min(MAX_S, left)
            chunks.append((g, s, row, col))
            row += s * SUB
            col += s * 8
            left -= s
    return chunks


BUFS = int(os.environ.get("KBUFS", "3"))
PREF = int(os.environ.get("KPREF", "1"))


def _build_program(chunks, n_users, n_items):
    nc = bacc.Bacc("TRN2", target_bir_lowering=False, debug=False,
                   enable_asserts=False)
    padtot = sum(s for (_, s, _, _) in chunks) * SUB
    icols = padtot // 16

    rt_d = nc.dram_tensor("rt", [64, padtot], F32, kind="ExternalInput")
    uidx_d = nc.dram_tensor("uidx", [128, icols], I16, kind="ExternalInput")
    iidx_d = nc.dram_tensor("iidx", [128, icols], I16, kind="ExternalInput")
    tblu_d = nc.dram_tensor("tblu", [n_users, D], F32, kind="ExternalInput")
    tbli_d = nc.dram_tensor("tbli", [n_items, D], F32, kind="ExternalInput")
    w1_d = nc.dram_tensor("w1", [64, 64], F32, kind="ExternalInput")
    w2_d = nc.dram_tensor("w2p", [64, 64], F32, kind="ExternalInput")
    w3_d = nc.dram_tensor("w3p", [64, 64], F32, kind="ExternalInput")
    out_d = nc.dram_tensor("out2", [64, padtot], F32, kind="ExternalOutput")

    T = len(chunks)

    with tile.TileContext(nc) as tc:
        tc._drain_and_barrier = types.MethodType(_split_drain_and_barrier, tc)
        with tc.tile_pool(name="const", bufs=1) as constp, \
             tc.tile_pool(name="rtp", bufs=BUFS) as rtp, \
             tc.tile_pool(name="gup", bufs=BUFS) as gup, \
             tc.tile_pool(name="gip", bufs=BUFS) as gip, \
             tc.tile_pool(name="outp", bufs=BUFS) as outp, \
             tc.tile_pool(name="utp", bufs=BUFS) as utp, \
             tc.tile_pool(name="itp", bufs=BUFS) as itp, \
             tc.tile_pool(name="scr", bufs=1, space="PSUM") as scrp, \
             tc.tile_pool(name="tpp", bufs=4, space="PSUM") as tpp, \
             tc.tile_pool(name="mmp", bufs=3, space="PSUM") as mmp:

            ident = constp.tile([128, 128], F32)
            make_identity(nc, ident[:])
            w1_t = constp.tile([64, 64], F32)
            nc.sync.dma_start(out=w1_t[:], in_=w1_d.ap()[:, :])
            w2_t = constp.tile([64, 64], F32)
            nc.sync.dma_start(out=w2_t[:], in_=w2_d.ap()[:, :])
            w3_t = constp.tile([64, 64], F32)
            nc.sync.dma_start(out=w3_t[:], in_=w3_d.ap()[:, :])
            uidx_t = constp.tile([128, icols], I16)
            nc.sync.dma_start(out=uidx_t[:], in_=uidx_d.ap()[:, :])
            iidx_t = constp.tile([128, icols], I16)
            nc.sync.dma_start(out=iidx_t[:], in_=iidx_d.ap()[:, :])
            dummy_sb = constp.tile([64, 128], F32)
            pscr = constp.tile([16, 16], I16)

            scratch = scrp.tile([64, 128], F32)
            # PE warmups: observe the identity (Pool) and weight-load (HWDGE)
            # semaphores with one wait each.
            nc.tensor.transpose(out=scratch[:], in_=ident[:, 0:64],
                                identity=ident[:])
            nc.tensor.matmul(out=scratch[:, 0:64], lhsT=w2_t[:],
                             rhs=w2_t[:], start=True, stop=True)
            nc.tensor.matmul(out=scratch[:, 0:64], lhsT=w3_t[:],
                             rhs=w3_t[:], start=True, stop=True)
            nc.tensor.matmul(out=scratch[:, 0:64], lhsT=w1_t[:],
                             rhs=w1_t[:], start=True, stop=True)
            # Pool warmups: observe the index-table loads.
            nc.gpsimd.tensor_copy(out=pscr[:, :], in_=uidx_t[0:16, 0:16])
            nc.gpsimd.tensor_copy(out=pscr[:, :], in_=iidx_t[0:16, 0:16])

            nreg = {}
            for (_, s, _, _) in chunks:
                if s not in nreg:
                    nreg[s] = nc.gpsimd.to_reg(s * SUB)

            ubase = [g // 2 * TCH for g in range(8)]
            usize = [min(TCH, n_users - b) for b in ubase]
            ibase = [g % 2 * TCH for g in range(8)]
            isize = [min(TCH, n_items - b) for b in ibase]

            rt_tiles = [None] * T
            gu_tiles = [None] * T
            gi_tiles = [None] * T
            ui_tiles = [None] * T
            ps_tiles = [None] * T
            o_tiles = [None] * T

            def issue_loads(t):
                g, s, row, col = chunks[t]
                rt_t = rtp.tile([64, MAX_S * 128], F32, tag="rt")
                nc.sync.dma_start(
                    out=rt_t[:, :s * 128],
                    in_=rt_d.ap()[:, row: row + s * 128])
                gu_t = gup.tile([128, MAX_S * 64], F32, tag="gu")
                nc.gpsimd.dma_gather(
                    out_ap=gu_t[:, :s * 64].rearrange("p (n d) -> p n d", d=64),
                    in_ap=tblu_d.ap()[ubase[g]:ubase[g] + usize[g], :],
                    idxs_ap=uidx_t[:, col:col + s * 8],
                    num_idxs=s * SUB, num_idxs_reg=nreg[s], elem_size=64)
                gi_t = gip.tile([128, MAX_S * 64], F32, tag="gi")
                nc.gpsimd.dma_gather(
                    out_ap=gi_t[:, :s * 64].rearrange("p (n d) -> p n d", d=64),
                    in_ap=tbli_d.ap()[ibase[g]:ibase[g] + isize[g], :],
                    idxs_ap=iidx_t[:, col:col + s * 8],
                    num_idxs=s * SUB, num_idxs_reg=nreg[s], elem_size=64)
                rt_tiles[t], gu_tiles[t], gi_tiles[t] = rt_t, gu_t, gi_t

            def issue_transposes(t):
                _, s, _, _ = chunks[t]
                gu_t, gi_t = gu_tiles[t], gi_tiles[t]
                if os.environ.get("KHDR", "0") == "1":
                    # Discarded header transposes absorb the two gather waits.
                    nc.tensor.transpose(out=scratch[:], in_=gu_t[:, 0:64],
                                        identity=ident[:])
                    nc.tensor.transpose(out=scratch[:], in_=gi_t[:, 0:64],
                                        identity=ident[:])
                ut_t = utp.tile([64, MAX_S * 128], F32, tag="ut")
                it_t = itp.tile([64, MAX_S * 128], F32, tag="it")
                for g4 in range((s + 3) // 4):
                    w = min(4, s - g4 * 4)
                    tpu = tpp.tile([64, 512], F32, tag="tp")
                    for jj in range(w):
                        j = g4 * 4 + jj
                        nc.tensor.transpose(
                            out=tpu[:, jj * 128:(jj + 1) * 128],
                            in_=gu_t[:, j * 64:(j + 1) * 64],
                            identity=ident[:])
                    nc.vector.tensor_copy(
                        out=ut_t[:, g4 * 512:g4 * 512 + w * 128],
                        in_=tpu[:, :w * 128])
                    tpi = tpp.tile([64, 512], F32, tag="tp")
                    for jj in range(w):
                        j = g4 * 4 + jj
                        nc.tensor.transpose(
                            out=tpi[:, jj * 128:(jj + 1) * 128],
                            in_=gi_t[:, j * 64:(j + 1) * 64],
                            identity=ident[:])
                    nc.vector.tensor_copy(
                        out=it_t[:, g4 * 512:g4 * 512 + w * 128],
                        in_=tpi[:, :w * 128])
                ui_tiles[t] = (ut_t, it_t)

            def issue_matmuls(t):
                _, s, _, _ = chunks[t]
                n = s * 64
                pss = []
                rt_t = rt_tiles[t]
                ut_t, it_t = ui_tiles[t]
                for q in range(2):
                    ps = mmp.tile([64, 512], F32, tag="mm")
                    ps_s = ps[:, :n]
                    nc.tensor.matmul(out=ps_s, lhsT=w2_t[:],
                                     rhs=ut_t[:, q * n:(q + 1) * n],
                                     start=True, stop=False)
                    nc.tensor.matmul(out=ps_s, lhsT=w3_t[:],
                                     rhs=it_t[:, q * n:(q + 1) * n],
                                     start=False, stop=False)
                    nc.tensor.matmul(out=ps_s, lhsT=w1_t[:],
                                     rhs=rt_t[:, q * n:(q + 1) * n],
                                     start=False, stop=True)
                    pss.append(ps)
                ps_tiles[t] = pss

            def issue_relus(t):
                _, s, _, _ = chunks[t]
                n = s * 64
                pss = ps_tiles[t]
                o_t = outp.tile([64, MAX_S * 128], F32, tag="o")
                for q in range(2):
                    nc.vector.tensor_scalar_max(
                        out=o_t[:, q * n:(q + 1) * n],
                        in0=pss[q][:, :n], scalar1=0.0)
                o_tiles[t] = o_t

            def issue_store(t):
                _, s, row, _ = chunks[t]
                nc.sync.dma_start(
                    out=out_d.ap()[:, row: row + s * 128],
                    in_=o_tiles[t][:, :s * 128])

            # Software-pipelined emission (see module docstring).
            for tt in range(min(PREF, T)):
                issue_loads(tt)
            issue_transposes(0)
            for t in range(T):
                if t + PREF < T:
                    issue_loads(t + PREF)
                issue_matmuls(t)
                if t + 1 < T:
                    issue_transposes(t + 1)
                else:
                    # Dummy PE op after the last matmuls + a DVE observer so
                    # the last relus elide their PE wait.
                    nc.tensor.transpose(out=scratch[:], in_=ident[:, 0:64],
                                        identity=ident[:])
                    nc.vector.tensor_copy(out=dummy_sb[:], in_=scratch[:])
                issue_relus(t)
                issue_store(t)
    nc.finalize()
    return nc


_PROGRAM_CACHE: dict = {}


def _get_program(chunk_key, n_users, n_items):
    key = (chunk_key, n_users, n_items)
    if key not in _PROGRAM_CACHE:
        _PROGRAM_CACHE[key] = (
            _build_program(_chunk_list(list(chunk_key)), n_users, n_items))
    return _PROGRAM_CACHE[key]


def _pack_rt(rv_sorted, chunks):
    """[PADTOT, 64] sorted/padded reviews -> [64, PADTOT] feature-major."""
    return np.ascontiguousarray(rv_sorted.T)


def _unpack_out(o2, chunks):
    """[64, PADTOT] transposed output -> [PADTOT, 64]."""
    return np.ascontiguousarray(o2.T)


def _wrap_idx(flat_sorted, chunks):
    """Rebased int16 indices [PADTOT] -> [128, PADTOT//16] in dma_gather's
    wrapped layout: per chunk block [128, 8*s] with block[p, m] =
    flat[m*16 + p%16], replicated across the 8 16-partition groups."""
    cols = []
    for (_, s, row, _) in chunks:
        blk = flat_sorted[row:row + s * SUB].reshape(s * 8, 16).T  # [16, 8s]
        cols.append(np.tile(blk, (8, 1)))
    return np.ascontiguousarray(np.concatenate(cols, axis=1))


def _run(review_vecs, user_vecs, item_vecs, W,
         review_user_adj, review_item_adj, perm_u, perm_i,
         n_cores, rpc):
    n_users = user_vecs.shape[0]
    n_items = item_vecs.shape[0]

    W = np.asarray(W, np.float32)
    W1 = np.ascontiguousarray(W[0:64])
    W2 = W[64:128]
    W3 = W[128:192]
    perm_u = np.asarray(perm_u, np.int64)
    perm_i = np.asarray(perm_i, np.int64)
    W2p = np.empty_like(W2)
    W2p[perm_u] = W2
    W3p = np.empty_like(W3)
    W3p[perm_i] = W3
    W2p = np.ascontiguousarray(W2p)
    W3p = np.ascontiguousarray(W3p)

    user_vecs = np.ascontiguousarray(np.asarray(user_vecs, np.float32))
    item_vecs = np.ascontiguousarray(np.asarray(item_vecs, np.float32))
    review_vecs = np.asarray(review_vecs, np.float32)
    au_all = np.asarray(review_user_adj, np.int64)
    ai_all = np.asarray(review_item_adj, np.int64)

    # Group each core's reviews by (user 32K chunk, item 32K chunk).
    per_core = []
    s_max = np.zeros(8, np.int64)
    for c in range(n_cores):
        lo = c * rpc
        au = au_all[lo:lo + rpc]
        ai = ai_all[lo:lo + rpc]
        grp = (au // TCH) * 2 + (ai // TCH)
        order = np.argsort(grp, kind="stable")
        counts = np.bincount(grp, minlength=8)
        per_core.append((order, counts))
        s_max = np.maximum(s_max, -(-counts // SUB))
    # shared chunk structure: even sub-tile counts per group
    s_per_group = [int(s + (s % 2)) for s in s_max]
    chunk_key = tuple(s_per_group)
    chunks = _chunk_list(s_per_group)
    padtot = sum(s for (_, s, _, _) in chunks) * SUB

    nc = _get_program(chunk_key, n_users, n_items)

    in_maps = []
    slotmaps = []
    for c in range(n_cores):
        lo = c * rpc
        au = au_all[lo:lo + rpc]
        ai = ai_all[lo:lo + rpc]
        order, counts = per_core[c]
        grp_sorted_bounds = np.cumsum(counts)
        slotmap = np.full(padtot, -1, np.int64)
        row = 0
        start = 0
        for g in range(8):
            cnt = int(counts[g])
            ids = order[start:start + cnt]
            slotmap[row:row + cnt] = ids
            start += cnt
            row += s_per_group[g] * SUB
        valid = slotmap >= 0
        sl = np.where(valid, slotmap, 0)

        rv_sorted = np.where(valid[:, None],
                             review_vecs[lo:lo + rpc][sl], 0.0).astype(np.float32)
        slot_g = np.repeat(np.arange(8), np.array(s_per_group) * SUB)
        u_reb = np.where(valid, au[sl] - (slot_g // 2) * TCH, 0).astype(np.int16)
        i_reb = np.where(valid, ai[sl] - (slot_g % 2) * TCH, 0).astype(np.int16)

        in_maps.append({
            "rt": _pack_rt(rv_sorted, chunks),
            "uidx": _wrap_idx(u_reb, chunks),
            "iidx": _wrap_idx(i_reb, chunks),
            "tblu": user_vecs,
            "tbli": item_vecs,
            "w1": W1,
            "w2p": W2p,
            "w3p": W3p,
        })
        slotmaps.append((slotmap, valid))

    res = run_bass_kernel_spmd(nc, in_maps, core_ids=list(range(n_cores)))

    out = np.empty((n_cores * rpc, 64), np.float32)
    for c in range(n_cores):
        o2 = np.asarray(res.results[c]["out2"], np.float32)
        out_sorted = _unpack_out(o2, chunks)
        slotmap, valid = slotmaps[c]
        out[c * rpc + slotmap[valid]] = out_sorted[valid]
    return out


def kernel(**inputs) -> np.ndarray:
    return _run(
        inputs["review_vecs"], inputs["user_vecs"], inputs["item_vecs"],
        inputs["W"], inputs["review_user_adj"], inputs["review_item_adj"],
        inputs["perm_u"], inputs["perm_i"],
        n_cores=N_CORES, rpc=RPC)
